# revision 12
# baseline (speedup 1.0000x reference)
"""Fused attention layer (QKV -> softmax -> fc + residual -> LayerNorm) on 8 TRN2 cores.

Problem: x [4,4,2048,64] f32, per-(b,x)-pair attention with D=64, S=2048.
Sharding: pure data parallel over the 16 (b,x) pairs -> 2 pairs per core.

Device algorithm per pair (layouts chosen so no on-chip transposes of big
tensors are needed; x^T and W^T are prepared host-side):
  - xT replicated into both partition halves [128, 2048] so K=64 matmuls can
    be 2x row-packed (tile_position rows 0/64).
  - QT/KT [128, 2048]: produced in both halves via 2x col-packed matmuls.
  - V [2048, 64] tiles stored [128, 16, 65] with a ones column (col 64) so the
    context matmul also produces softmax denominators ("ones trick").
  - scoresT per (k-tile j, q-chunk c): KT_j.T @ QT_c -> bf16 psum, 2x
    row-packed (even j rows 0-63, odd j rows 64-127, different banks).
  - expT = exp(0.125 * scoresT): one ACT op per 4 k-tiles [128, 2048].
  - ctx: [V_j | 1].T @ expT_j -> psum [65, 512] accum; row 64 = sumexp.
  - r = 1/sumexp via 16 PE transposes [1,128]->[128,1] + DVE reciprocal.
  - fc: ctxT_chunk.T @ wfT -> [128 q, 64 f]; out_res = fc*r + x (fused STT).
  - LayerNorm: bn_stats/bn_aggr; rstd via Newton rsqrt on DVE (no ACT table
    switches -- Exp is the only ACT function in the kernel).
"""

import sys

sys.path.insert(0, "/opt/trn_rl_repo")

from contextlib import ExitStack

import numpy as np
import ml_dtypes

import concourse.bass as bass
import concourse.tile as tile
from concourse import bacc
from concourse import mybir
from concourse.bass import ts

F32 = mybir.dt.float32
BF16 = mybir.dt.bfloat16
AF = mybir.ActivationFunctionType
OP = mybir.AluOpType

PAIRS = 2        # (b,x) pairs per core
S = 2048         # sequence length
D = 64           # d_model
NT = S // 128    # 16 q/k tiles of 128
NC = S // 512    # 4 q-chunks of 512
EPS = 1e-5
# linear seed for Newton rsqrt, valid for var+eps in ~[0.34, 2.25]
RS_A = 1.5333296
RS_B = -0.4427810
import os
PACK_SCORES = os.environ.get('PACK_SCORES', '1') == '1'
PACK_V = os.environ.get('PACK_V', '1') == '1'
PACK_QK = os.environ.get('PACK_QK', '1') == '1'


def build_bass():
    nc = bacc.Bacc()
    x_d = nc.declare_dram_parameter("x", [PAIRS, S, D], F32, isOutput=False)
    xT_d = nc.declare_dram_parameter("xT", [PAIRS, D, S], BF16, isOutput=False)
    wT_d = nc.declare_dram_parameter("wT", [4, D, D], BF16, isOutput=False)
    out_d = nc.declare_dram_parameter("out", [PAIRS, S, D], F32, isOutput=True)

    with ExitStack() as ctx:
        tc = ctx.enter_context(tile.TileContext(nc))
        singles = ctx.enter_context(tc.tile_pool(name="singles", bufs=1))
        sb2 = ctx.enter_context(tc.tile_pool(name="sb2", bufs=2))
        expP = ctx.enter_context(tc.tile_pool(name="expP", bufs=2))
        lnP = ctx.enter_context(tc.tile_pool(name="lnP", bufs=4))
        sP = ctx.enter_context(tc.tile_pool(name="sP", bufs=3, space="PSUM"))
        ctxP = ctx.enter_context(tc.tile_pool(name="ctxP", bufs=1, space="PSUM"))
        mP = ctx.enter_context(tc.tile_pool(name="mP", bufs=1, space="PSUM"))

        # Constants: lo rows hold wqT|wkT|wvT|wfT, hi rows replicate wvT for
        # the row-packed V matmuls.
        w_sb = singles.tile([128, 4, D], BF16)
        nc.sync.dma_start(
            out=w_sb[0:D, :, :], in_=wT_d[:].rearrange("w d e -> d w e")
        )
        nc.sync.dma_start(out=w_sb[D : 2 * D, 2, :], in_=wT_d[2])
        id1 = singles.tile([128, 1], BF16)
        nc.vector.memset(id1, 1.0)

        for p in range(PAIRS):
            # ---- loads (xT broadcast into both partition halves) ----
            xT_sb = sb2.tile([128, S], BF16, tag="xT")
            nc.sync.dma_start(out=xT_sb[0:D, :], in_=xT_d[p])
            nc.sync.dma_start(out=xT_sb[D : 2 * D, :], in_=xT_d[p])
            x_sb = sb2.tile([128, NT, D], F32, tag="x")
            nc.sync.dma_start(
                out=x_sb, in_=x_d[p].rearrange("(t q) d -> q t d", q=128)
            )

            # ---- Q^T / K^T projections (2x col-packed: lo+hi copies) ----
            qT_sb = sb2.tile([128, S], BF16, tag="qT")
            kT_sb = sb2.tile([128, S], BF16, tag="kT")
            for c in range(NC):
                k_ps = mP.tile([128, 512], F32, tag="m")
                for h in range(2 if PACK_QK else 1):
                    nc.tensor.matmul(
                        k_ps[h * D : (h + 1) * D, :], lhsT=w_sb[0:D, 1, :],
                        rhs=xT_sb[0:D, ts(c, 512)], start=True, stop=True,
                        tile_position=(0, h * D),
                    )
                nc.vector.tensor_copy(out=kT_sb[:, ts(c, 512)], in_=k_ps)

            # ---- V (2x row-packed; ones column at e=64) ----
            v_sb = sb2.tile([128, NT, D + 1], BF16, tag="v")
            nc.vector.memset(v_sb[:, :, D : D + 1], 1.0)
            v_psA = mP.tile([128, 512], F32, tag="m")
            v_psB = mP.tile([128, 512], F32, tag="m")
            for j2 in range(8):
                j0, j1 = 2 * j2, 2 * j2 + 1
                nc.tensor.matmul(
                    v_psA[:, ts(j2, D)], lhsT=xT_sb[0:D, ts(j0, 128)],
                    rhs=w_sb[0:D, 2, :], start=True, stop=True,
                    tile_position=(0, 0),
                )
                if PACK_V:
                    nc.tensor.matmul(
                        v_psB[:, ts(j2, D)], lhsT=xT_sb[D : 2 * D, ts(j1, 128)],
                        rhs=w_sb[D : 2 * D, 2, :], start=True, stop=True,
                        tile_position=(D, 0),
                    )
                else:
                    nc.tensor.matmul(
                        v_psB[:, ts(j2, D)], lhsT=xT_sb[0:D, ts(j1, 128)],
                        rhs=w_sb[0:D, 2, :], start=True, stop=True,
                        tile_position=(0, 0),
                    )
            nc.vector.tensor_copy(
                out=v_sb[:, 0 : NT : 2, 0:D],
                in_=v_psA[:].rearrange("p (a b) -> p a b", a=8),
            )
            nc.vector.tensor_copy(
                out=v_sb[:, 1 : NT : 2, 0:D],
                in_=v_psB[:].rearrange("p (a b) -> p a b", a=8),
            )

            # ---- attention: scores (2x row-packed) -> exp -> ctx; fc/LN per chunk ----
            ctxT_sb = sb2.tile([D + 1, NC, 512], BF16, tag="ctxT")
            r_sb = lnP.tile([128, NT], F32, tag="r")
            out_res = sb2.tile([128, NT, D], F32, tag="ores")
            mv = lnP.tile([128, NT, 2], F32, tag="mv")
            for c in range(NC):
                q_ps = mP.tile([128, 512], F32, tag="m")
                for h in range(2 if PACK_QK else 1):
                    nc.tensor.matmul(
                        q_ps[h * D : (h + 1) * D, :], lhsT=w_sb[0:D, 0, :],
                        rhs=xT_sb[0:D, ts(c, 512)], start=True, stop=True,
                        tile_position=(0, h * D),
                    )
                nc.vector.tensor_copy(out=qT_sb[:, ts(c, 512)], in_=q_ps)
                expT = expP.tile([128, NT, 512], BF16, tag="expT")
                ctx_ps = ctxP.tile([128, 512], F32, tag="ctx")
                for b in range(NT // 2):  # batches of 2 k-tiles (row-packed pair)
                    j0, j1 = 2 * b, 2 * b + 1
                    s_ps = sP.tile([128, 1024], F32, tag="s")
                    # even j -> rows 0-63 (bank A), odd j -> rows 64-127 (bank B)
                    nc.tensor.matmul(
                        s_ps[:, 0:512], lhsT=kT_sb[0:D, ts(j0, 128)],
                        rhs=qT_sb[0:D, ts(c, 512)],
                        start=True, stop=True, tile_position=(0, 0),
                    )
                    if PACK_SCORES:
                        nc.tensor.matmul(
                            s_ps[:, 512:1024], lhsT=kT_sb[D : 2 * D, ts(j1, 128)],
                            rhs=qT_sb[D : 2 * D, ts(c, 512)],
                            start=True, stop=True, tile_position=(D, 0),
                        )
                    else:
                        nc.tensor.matmul(
                            s_ps[:, 512:1024], lhsT=kT_sb[0:D, ts(j1, 128)],
                            rhs=qT_sb[0:D, ts(c, 512)],
                            start=True, stop=True, tile_position=(0, 0),
                        )
                    nc.scalar.activation(
                        out=expT[:, j0 : j0 + 2, :],
                        in_=s_ps[:].rearrange("p (a f) -> p a f", a=2),
                        func=AF.Exp, scale=0.125,
                    )
                    for j in (j0, j1):
                        nc.tensor.matmul(
                            ctx_ps[0 : D + 1, :], lhsT=v_sb[:, j, :],
                            rhs=expT[:, j, :], start=(j == 0), stop=(j == NT - 1),
                        )
                nc.vector.tensor_copy(
                    out=ctxT_sb[:, c, :], in_=ctx_ps[0 : D + 1, :]
                )

                # sums -> r for this chunk's 4 q-tiles
                r_ps = mP.tile([128, 8], BF16, tag="m")
                for w in range(4):
                    nc.tensor.transpose(
                        out=r_ps[:, 2 * w : 2 * w + 1],
                        in_=ctxT_sb[D : D + 1, c, ts(w, 128)],
                        identity=id1[D : D + 1, 0:1],
                    )
                nc.vector.reciprocal(
                    out=r_sb[:, 4 * c : 4 * c + 4], in_=r_ps[:, 0:8:2]
                )

                # fc + residual + LN stats for this chunk's 4 q-tiles
                for w in range(4):
                    t = 4 * c + w
                    fc_ps = mP.tile([128, 512], F32, tag="m")
                    nc.tensor.matmul(
                        fc_ps[:, 0:D], lhsT=ctxT_sb[0:D, c, ts(w, 128)],
                        rhs=w_sb[0:D, 3, :], start=True, stop=True,
                    )
                    nc.vector.scalar_tensor_tensor(
                        out=out_res[:, t, :], in0=fc_ps[:, 0:D],
                        scalar=r_sb[:, t : t + 1], in1=x_sb[:, t, :],
                        op0=OP.mult, op1=OP.add,
                    )
                    bst = lnP.tile([128, 6], F32, tag="bst")
                    nc.vector.bn_stats(out=bst, in_=out_res[:, t, :])
                    nc.vector.bn_aggr(out=mv[:, t, :], in_=bst)

            # ---- rstd via Newton rsqrt on DVE (3 iterations) ----
            ve = lnP.tile([128, NT], F32, tag="ve")
            nc.vector.tensor_scalar(
                out=ve, in0=mv[:, :, 1], scalar1=EPS, scalar2=None, op0=OP.add
            )
            rstd = lnP.tile([128, NT], F32, tag="rstd")
            nc.vector.tensor_scalar(
                out=rstd, in0=ve, scalar1=RS_B, scalar2=RS_A,
                op0=OP.mult, op1=OP.add,
            )
            t2 = lnP.tile([128, NT], F32, tag="t2")
            for _ in range(3):
                nc.vector.tensor_mul(t2, rstd, rstd)
                nc.vector.scalar_tensor_tensor(
                    out=t2, in0=t2, scalar=-0.5, in1=ve,
                    op0=OP.mult, op1=OP.mult,
                )
                nc.vector.tensor_scalar(
                    out=t2, in0=t2, scalar1=1.5, scalar2=None, op0=OP.add
                )
                nc.vector.tensor_mul(rstd, rstd, t2)

            out_sb = sb2.tile([128, NT, D], F32, tag="osb")
            for t in range(NT):
                nc.vector.tensor_scalar(
                    out=out_sb[:, t, :], in0=out_res[:, t, :],
                    scalar1=mv[:, t, 0:1], scalar2=rstd[:, t : t + 1],
                    op0=OP.subtract, op1=OP.mult,
                )
            nc.sync.dma_start(
                out=out_d[p].rearrange("(t q) d -> q t d", q=128), in_=out_sb
            )
    nc.compile()
    return nc


_NC_CACHE = None


def _get_nc():
    global _NC_CACHE
    if _NC_CACHE is None:
        _NC_CACHE = build_bass()
    return _NC_CACHE


def _make_in_maps(x, W_Q, W_K, W_V, W_fc):
    xf = np.ascontiguousarray(np.asarray(x, dtype=np.float32).reshape(16, S, D))
    wT = np.ascontiguousarray(
        np.stack([np.asarray(w, dtype=np.float32).T for w in (W_Q, W_K, W_V, W_fc)])
    ).astype(ml_dtypes.bfloat16)
    in_maps = []
    for i in range(8):
        xs = xf[2 * i : 2 * i + 2]
        xTs = np.ascontiguousarray(xs.transpose(0, 2, 1)).astype(ml_dtypes.bfloat16)
        in_maps.append({"x": xs, "xT": xTs, "wT": wT})
    return in_maps


def run(x, W_Q, W_K, W_V, W_fc, trace=False):
    from concourse.bass_utils import run_bass_kernel_spmd

    res = run_bass_kernel_spmd(
        _get_nc(),
        _make_in_maps(x, W_Q, W_K, W_V, W_fc),
        core_ids=list(range(8)),
        trace=trace,
    )
    out = np.concatenate([r["out"] for r in res.results], axis=0)
    return out.reshape(4, 4, S, D).astype(np.float32), res


def kernel(x, W_Q, W_K, W_V, W_fc):
    out, _ = run(x, W_Q, W_K, W_V, W_fc)
    return out


# revision 13
# speedup vs baseline: 1.3276x; 1.3276x over previous
"""Fused attention layer (QKV -> softmax -> fc + residual -> LayerNorm) on 8 TRN2 cores.

Problem: x [4,4,2048,64] f32, per-(b,x)-pair attention with D=64, S=2048.
Sharding: pure data parallel over the 16 (b,x) pairs -> 2 pairs per core.

Device algorithm per pair (layouts chosen so no on-chip transposes of big
tensors are needed; x^T and W^T are prepared host-side):
  - xT replicated into both partition halves [128, 2048] so K=64 matmuls can
    be 2x row-packed (tile_position rows 0/64).
  - QT/KT [128, 2048]: produced in both halves via 2x col-packed matmuls.
  - V [2048, 64] tiles stored [128, 16, 65] with a ones column (col 64) so the
    context matmul also produces softmax denominators ("ones trick").
  - scoresT per (k-tile j, q-chunk c): KT_j.T @ QT_c -> bf16 psum, 2x
    row-packed (even j rows 0-63, odd j rows 64-127, different banks).
  - expT = exp(0.125 * scoresT): one ACT op per 4 k-tiles [128, 2048].
  - ctx: [V_j | 1].T @ expT_j -> psum [65, 512] accum; row 64 = sumexp.
  - r = 1/sumexp via 16 PE transposes [1,128]->[128,1] + DVE reciprocal.
  - fc: ctxT_chunk.T @ wfT -> [128 q, 64 f]; out_res = fc*r + x (fused STT).
  - LayerNorm: bn_stats/bn_aggr; rstd via Newton rsqrt on DVE (no ACT table
    switches -- Exp is the only ACT function in the kernel).
"""

import sys

sys.path.insert(0, "/opt/trn_rl_repo")

from contextlib import ExitStack

import numpy as np
import ml_dtypes

import concourse.bass as bass
import concourse.tile as tile
from concourse import bacc
from concourse import mybir
from concourse.bass import ts

F32 = mybir.dt.float32
BF16 = mybir.dt.bfloat16
AF = mybir.ActivationFunctionType
OP = mybir.AluOpType

PAIRS = 2        # (b,x) pairs per core
S = 2048         # sequence length
D = 64           # d_model
NT = S // 128    # 16 q/k tiles of 128
NC = S // 512    # 4 q-chunks of 512
EPS = 1e-5
# linear seed for Newton rsqrt, valid for var+eps in ~[0.34, 2.25]
RS_A = 1.5333296
RS_B = -0.4427810
import os
PACK_SCORES = os.environ.get('PACK_SCORES', '1') == '1'
PACK_V = os.environ.get('PACK_V', '1') == '1'
PACK_QK = os.environ.get('PACK_QK', '1') == '1'


def build_bass():
    nc = bacc.Bacc()
    x_d = nc.declare_dram_parameter("x", [PAIRS, S, D], F32, isOutput=False)
    xT_d = nc.declare_dram_parameter("xT", [PAIRS, D, S], BF16, isOutput=False)
    wT_d = nc.declare_dram_parameter("wT", [4, D, D], BF16, isOutput=False)
    out_d = nc.declare_dram_parameter("out", [PAIRS, S, D], F32, isOutput=True)

    with ExitStack() as ctx:
        tc = ctx.enter_context(tile.TileContext(nc))
        singles = ctx.enter_context(tc.tile_pool(name="singles", bufs=1))
        sb2 = ctx.enter_context(tc.tile_pool(name="sb2", bufs=2))
        expP = ctx.enter_context(tc.tile_pool(name="expP", bufs=2))
        lnP = ctx.enter_context(tc.tile_pool(name="lnP", bufs=4))
        sP = ctx.enter_context(tc.tile_pool(name="sP", bufs=3, space="PSUM"))
        ctxP = ctx.enter_context(tc.tile_pool(name="ctxP", bufs=1, space="PSUM"))
        mP = ctx.enter_context(tc.tile_pool(name="mP", bufs=1, space="PSUM"))

        # Constants: lo rows hold wqT|wkT|wvT|wfT, hi rows replicate wvT for
        # the row-packed V matmuls.
        w_sb = singles.tile([128, 4, D], BF16)
        nc.sync.dma_start(
            out=w_sb[0:D, :, :], in_=wT_d[:].rearrange("w d e -> d w e")
        )
        nc.sync.dma_start(out=w_sb[D : 2 * D, 2, :], in_=wT_d[2])
        id1 = singles.tile([128, 1], BF16)
        nc.vector.memset(id1, 1.0)

        for p in range(PAIRS):
            # ---- loads (xT broadcast into both partition halves) ----
            xT_sb = sb2.tile([128, S], BF16, tag="xT")
            nc.sync.dma_start(out=xT_sb[0:D, :], in_=xT_d[p])
            nc.sync.dma_start(out=xT_sb[D : 2 * D, :], in_=xT_d[p])
            x_sb = sb2.tile([128, NT, D], F32, tag="x")
            nc.sync.dma_start(
                out=x_sb, in_=x_d[p].rearrange("(t q) d -> q t d", q=128)
            )

            # ---- Q^T / K^T projections (2x col-packed: lo+hi copies) ----
            qT_sb = sb2.tile([128, S], BF16, tag="qT")
            kT_sb = sb2.tile([128, S], BF16, tag="kT")
            for c in range(NC):
                k_ps = mP.tile([128, 512], F32, tag="m")
                for h in range(2 if PACK_QK else 1):
                    nc.tensor.matmul(
                        k_ps[h * D : (h + 1) * D, :], lhsT=w_sb[0:D, 1, :],
                        rhs=xT_sb[0:D, ts(c, 512)], start=True, stop=True,
                        tile_position=(0, h * D),
                    )
                nc.vector.tensor_copy(out=kT_sb[:, ts(c, 512)], in_=k_ps)
                q_ps = mP.tile([128, 512], F32, tag="m")
                for h in range(2 if PACK_QK else 1):
                    nc.tensor.matmul(
                        q_ps[h * D : (h + 1) * D, :], lhsT=w_sb[0:D, 0, :],
                        rhs=xT_sb[0:D, ts(c, 512)], start=True, stop=True,
                        tile_position=(0, h * D),
                    )
                nc.vector.tensor_copy(out=qT_sb[:, ts(c, 512)], in_=q_ps)

            # ---- V (2x row-packed; ones column at e=64) ----
            v_sb = sb2.tile([128, NT, D + 1], BF16, tag="v")
            nc.vector.memset(v_sb[:, :, D : D + 1], 1.0)
            v_psA = mP.tile([128, 512], F32, tag="m")
            v_psB = mP.tile([128, 512], F32, tag="m")
            for j2 in range(8):
                j0, j1 = 2 * j2, 2 * j2 + 1
                nc.tensor.matmul(
                    v_psA[:, ts(j2, D)], lhsT=xT_sb[0:D, ts(j0, 128)],
                    rhs=w_sb[0:D, 2, :], start=True, stop=True,
                    tile_position=(0, 0),
                )
                if PACK_V:
                    nc.tensor.matmul(
                        v_psB[:, ts(j2, D)], lhsT=xT_sb[D : 2 * D, ts(j1, 128)],
                        rhs=w_sb[D : 2 * D, 2, :], start=True, stop=True,
                        tile_position=(D, 0),
                    )
                else:
                    nc.tensor.matmul(
                        v_psB[:, ts(j2, D)], lhsT=xT_sb[0:D, ts(j1, 128)],
                        rhs=w_sb[0:D, 2, :], start=True, stop=True,
                        tile_position=(0, 0),
                    )
            nc.vector.tensor_copy(
                out=v_sb[:, 0 : NT : 2, 0:D],
                in_=v_psA[:].rearrange("p (a b) -> p a b", a=8),
            )
            nc.vector.tensor_copy(
                out=v_sb[:, 1 : NT : 2, 0:D],
                in_=v_psB[:].rearrange("p (a b) -> p a b", a=8),
            )

            # ---- attention: scores (2x row-packed) -> exp -> ctx; fc/LN per chunk ----
            ctxT_sb = sb2.tile([D + 1, NC, 512], BF16, tag="ctxT")
            r_sb = lnP.tile([128, NT], F32, tag="r")
            out_res = sb2.tile([128, NT, D], F32, tag="ores")
            mv = lnP.tile([128, NT, 2], F32, tag="mv")
            for c in range(NC):
                expT = expP.tile([128, NT, 512], BF16, tag="expT")
                ctx_ps = ctxP.tile([128, 512], F32, tag="ctx")
                for b in range(NT // 2):  # batches of 2 k-tiles (row-packed pair)
                    j0, j1 = 2 * b, 2 * b + 1
                    s_ps = sP.tile([128, 1024], F32, tag="s")
                    # even j -> rows 0-63 (bank A), odd j -> rows 64-127 (bank B)
                    nc.tensor.matmul(
                        s_ps[:, 0:512], lhsT=kT_sb[0:D, ts(j0, 128)],
                        rhs=qT_sb[0:D, ts(c, 512)],
                        start=True, stop=True, tile_position=(0, 0),
                    )
                    if PACK_SCORES:
                        nc.tensor.matmul(
                            s_ps[:, 512:1024], lhsT=kT_sb[D : 2 * D, ts(j1, 128)],
                            rhs=qT_sb[D : 2 * D, ts(c, 512)],
                            start=True, stop=True, tile_position=(D, 0),
                        )
                    else:
                        nc.tensor.matmul(
                            s_ps[:, 512:1024], lhsT=kT_sb[0:D, ts(j1, 128)],
                            rhs=qT_sb[0:D, ts(c, 512)],
                            start=True, stop=True, tile_position=(0, 0),
                        )
                    nc.scalar.activation(
                        out=expT[:, j0 : j0 + 2, :],
                        in_=s_ps[:].rearrange("p (a f) -> p a f", a=2),
                        func=AF.Exp, scale=0.125,
                    )
                    for j in (j0, j1):
                        nc.tensor.matmul(
                            ctx_ps[0 : D + 1, :], lhsT=v_sb[:, j, :],
                            rhs=expT[:, j, :], start=(j == 0), stop=(j == NT - 1),
                        )
                nc.vector.tensor_copy(
                    out=ctxT_sb[:, c, :], in_=ctx_ps[0 : D + 1, :]
                )

                # sums -> r for this chunk's 4 q-tiles
                r_ps = mP.tile([128, 8], BF16, tag="m")
                for w in range(4):
                    nc.tensor.transpose(
                        out=r_ps[:, 2 * w : 2 * w + 1],
                        in_=ctxT_sb[D : D + 1, c, ts(w, 128)],
                        identity=id1[D : D + 1, 0:1],
                    )
                nc.vector.reciprocal(
                    out=r_sb[:, 4 * c : 4 * c + 4], in_=r_ps[:, 0:8:2]
                )

                # fc + residual + LN stats for this chunk's 4 q-tiles
                for w in range(4):
                    t = 4 * c + w
                    fc_ps = mP.tile([128, 512], F32, tag="m")
                    nc.tensor.matmul(
                        fc_ps[:, 0:D], lhsT=ctxT_sb[0:D, c, ts(w, 128)],
                        rhs=w_sb[0:D, 3, :], start=True, stop=True,
                    )
                    nc.vector.scalar_tensor_tensor(
                        out=out_res[:, t, :], in0=fc_ps[:, 0:D],
                        scalar=r_sb[:, t : t + 1], in1=x_sb[:, t, :],
                        op0=OP.mult, op1=OP.add,
                    )
                    bst = lnP.tile([128, 6], F32, tag="bst")
                    nc.vector.bn_stats(out=bst, in_=out_res[:, t, :])
                    nc.vector.bn_aggr(out=mv[:, t, :], in_=bst)

            # ---- rstd via Newton rsqrt on DVE (3 iterations) ----
            ve = lnP.tile([128, NT], F32, tag="ve")
            nc.vector.tensor_scalar(
                out=ve, in0=mv[:, :, 1], scalar1=EPS, scalar2=None, op0=OP.add
            )
            rstd = lnP.tile([128, NT], F32, tag="rstd")
            nc.vector.tensor_scalar(
                out=rstd, in0=ve, scalar1=RS_B, scalar2=RS_A,
                op0=OP.mult, op1=OP.add,
            )
            t2 = lnP.tile([128, NT], F32, tag="t2")
            for _ in range(3):
                nc.vector.tensor_mul(t2, rstd, rstd)
                nc.vector.scalar_tensor_tensor(
                    out=t2, in0=t2, scalar=-0.5, in1=ve,
                    op0=OP.mult, op1=OP.mult,
                )
                nc.vector.tensor_scalar(
                    out=t2, in0=t2, scalar1=1.5, scalar2=None, op0=OP.add
                )
                nc.vector.tensor_mul(rstd, rstd, t2)

            out_sb = sb2.tile([128, NT, D], F32, tag="osb")
            for t in range(NT):
                nc.vector.tensor_scalar(
                    out=out_sb[:, t, :], in0=out_res[:, t, :],
                    scalar1=mv[:, t, 0:1], scalar2=rstd[:, t : t + 1],
                    op0=OP.subtract, op1=OP.mult,
                )
            nc.sync.dma_start(
                out=out_d[p].rearrange("(t q) d -> q t d", q=128), in_=out_sb
            )
    nc.compile()
    return nc


_NC_CACHE = None


def _get_nc():
    global _NC_CACHE
    if _NC_CACHE is None:
        _NC_CACHE = build_bass()
    return _NC_CACHE


def _make_in_maps(x, W_Q, W_K, W_V, W_fc):
    xf = np.ascontiguousarray(np.asarray(x, dtype=np.float32).reshape(16, S, D))
    wT = np.ascontiguousarray(
        np.stack([np.asarray(w, dtype=np.float32).T for w in (W_Q, W_K, W_V, W_fc)])
    ).astype(ml_dtypes.bfloat16)
    in_maps = []
    for i in range(8):
        xs = xf[2 * i : 2 * i + 2]
        xTs = np.ascontiguousarray(xs.transpose(0, 2, 1)).astype(ml_dtypes.bfloat16)
        in_maps.append({"x": xs, "xT": xTs, "wT": wT})
    return in_maps


def run(x, W_Q, W_K, W_V, W_fc, trace=False):
    from concourse.bass_utils import run_bass_kernel_spmd

    res = run_bass_kernel_spmd(
        _get_nc(),
        _make_in_maps(x, W_Q, W_K, W_V, W_fc),
        core_ids=list(range(8)),
        trace=trace,
    )
    out = np.concatenate([r["out"] for r in res.results], axis=0)
    return out.reshape(4, 4, S, D).astype(np.float32), res


def kernel(x, W_Q, W_K, W_V, W_fc):
    out, _ = run(x, W_Q, W_K, W_V, W_fc)
    return out
